# revision 23
# baseline (speedup 1.0000x reference)
"""Trainium2 Bass kernel for 3-layer CuGraphSAGE on a fanout-8 sampled tree.

The sampled graph produced by fanout-based neighbor sampling is a forest of
B=4096 independent trees (children of parent p are rows [4096+8p, 4096+8p+8)).
We shard by seed block: core c gets 512 seeds plus their full 3-hop subtrees
(4 contiguous row blocks of x, exactly 1/8 of all rows, zero halo).

Per-core pipeline (all activations channel-major [128ch, rows] so the matmul
contraction dim is always the partition dim — no transposes on device):
  hop3 (87.5% of bytes, only ever aggregated) streams as fp8_e4m3 — the
  8-way mean dilutes its 3.6% quantization RMS to ~1e-3 of the final
  output.  It is de-interleaved per 512-parent group on the host (col
  e*512+p = child e of parent p), so the mean-aggregation is accumulating
  matmuls with CONTIGUOUS fp8 rhs slices (stride-8 APs stream ~5x slower
  on the PE); with GNN_DR=1 pairs of sibling planes go through fp8
  DoubleRow matmuls (2 MACs/cell/cycle), halving PE streaming time.
  hop2 is resident fp8 (self-features quantize at 3.6% RMS -> ~1e-2
  final); hop0+hop1 (whose rows dominate the output) stay bf16.  The
  small hop1/hop2 and layer-1/2 aggregations run as VectorE reduce_sum
  (DVE is otherwise idle) followed by one matmul, software-pipelined so
  they hide inside the big fp8 u-blocks.  DMA count is minimized (per-
  transfer overhead ~3us is serial): one weights+bias transfer, one
  hop0/1, two hop2, eight 4 MiB hop3, one staged output write-back.
  The 1/8 mean is folded into the aggregation weight (via the activation
  scale on the DoubleRow path, whose fp8 weights are built on-device);
  bias+ReLU on ScalarE evicts PSUM->SBUF in bf16.  h1/h2 live entirely
  in SBUF; the bf16 output is upcast to fp32 on the host.
"""

import os
import numpy as np

# ---------------------------------------------------------------- constants
N_CORES = 8
C = 128                       # channels
B = 4096                      # seeds
S = B // N_CORES              # 512 seeds per core
BLK = [512, 4096, 32768, 262144]          # per-core rows per hop
OFF = [0, 4096, 36864, 299008]            # global start row of each hop block
NPAR0 = BLK[0] + BLK[1] + BLK[2]          # 37376 local layer-0 parents
NPAR1 = BLK[0] + BLK[1]                   # 4608 local layer-1 parents
NH2 = BLK[2]                              # 32768 local hop2 rows
N3 = BLK[3]                               # 262144 local hop3 rows
PT = 512                                  # parents per PSUM tile
N_FULL = 2396160
E_FULL = 2392064
OUT_ROWS = 36864
DR_SCALE = 16.0               # fp8 DoubleRow agg-weight scale (see below)

TRACE = os.environ.get("GNN_TRACE", "0") == "1"
DR = os.environ.get("GNN_DR", "1") == "1"
H2F8 = os.environ.get("GNN_H2F8", "1") == "1"
LAST_RESULT = None

_BASS_CACHE = {}


def _build_bass(dr, h2f8):
    import concourse.mybir as mybir
    from concourse import bacc
    from concourse.tile import TileContext

    bf16 = mybir.dt.bfloat16
    fp8 = mybir.dt.float8e4
    f32 = mybir.dt.float32
    h2dt = fp8 if h2f8 else bf16
    Relu = mybir.ActivationFunctionType.Relu
    Ident = mybir.ActivationFunctionType.Identity
    AxX = mybir.AxisListType.X
    DRow = mybir.MatmulPerfMode.DoubleRow

    # Bacc (not raw Bass): its compile() pipeline splits multi-sem sync
    # waits into event semaphores — TRN2 allows at most 1 wait/instruction.
    nc = bacc.Bacc()
    xA = nc.dram_tensor("xA", [C, NPAR1], bf16, kind="ExternalInput")
    xh2d = nc.dram_tensor("xh2", [C, NH2], h2dt, kind="ExternalInput")
    x3 = nc.dram_tensor("x3", [C, N3], fp8, kind="ExternalInput")
    # 8 weight blocks + 3 bias columns in one bf16 tensor -> one DMA
    # (per-transfer overhead is serial on the DMA timeline)
    wconsts = nc.dram_tensor("wconsts", [C, 8 * C + 3], bf16,
                             kind="ExternalInput")
    out = nc.dram_tensor("out", [C, NPAR1], bf16, kind="ExternalOutput")
    WIDX = {k: i for i, k in enumerate(
        ("w1a", "w1b", "w2a", "w2b", "w3a", "w3b", "w1bs", "w1a_raw"))}

    with TileContext(nc) as tc:
        with tc.tile_pool(name="const", bufs=1) as constp, \
             tc.tile_pool(name="keep", bufs=1) as keepp, \
             tc.tile_pool(name="dbuf", bufs=2) as dpool, \
             tc.tile_pool(name="hbuf", bufs=2) as hpool, \
             tc.tile_pool(name="a0buf", bufs=2) as a0p, \
             tc.tile_pool(name="a1buf", bufs=3) as a1p, \
             tc.tile_pool(name="ps", bufs=6, space="PSUM") as pp:

            wtile = constp.tile([C, 8 * C + 3], bf16, name="wtile")
            nc.sync.dma_start(wtile[:, :], wconsts[:, :])
            w = {k: wtile[:, C * i: C * (i + 1)] for k, i in WIDX.items()}
            bt = {f"b{i+1}": wtile[:, 8 * C + i: 8 * C + i + 1]
                  for i in range(3)}
            # fp8 DoubleRow aggregation weight, built on-device: two
            # interleaved copies of DR_SCALE * W1a^T (saves a DMA transfer)
            wdrt = constp.tile([C, 2 * C], fp8, name="wdrt")
            with nc.allow_low_precision(
                    reason="fp8 DoubleRow agg weights; 8-way mean dilutes "
                           "the 3.6% fp8 RMS below tolerance"):
                nc.scalar.activation(wdrt[:, 0:C], w["w1a_raw"], Ident,
                                     scale=DR_SCALE)
                nc.scalar.activation(wdrt[:, C:2 * C], w["w1a_raw"], Ident,
                                     scale=DR_SCALE)

            # hop2 chunk 1 is the only input besides weights that block 1's
            # u-matmuls need, so it goes ahead of everything else; xA0 and
            # the remaining hop2 chunks queue behind the first X3 chunk.
            xh2t = keepp.tile([C, NH2], h2dt, tag="xh2t")
            nc.sync.dma_start(xh2t[:, 0: 8 * PT], xh2d[:, 0: 8 * PT])
            xA0 = keepp.tile([C, NPAR1], bf16, tag="xA0")

            h1self = keepp.tile([C, NPAR1], bf16, tag="h1self")
            h2sb = keepp.tile([C, NPAR1], bf16, tag="h2sb")
            l2agg = keepp.tile([C, PT], bf16, tag="l2agg")
            ostage = keepp.tile([C, NPAR1], bf16, tag="ostage")

            def red8(dst_ap, children_ap):
                # dst[c, p] = sum_e children[c, 8p+e]  (natural node order)
                with nc.allow_low_precision(
                        reason="8-way sibling sum is fp32 internal on DVE; "
                               "bf16 rounding of the sum is within tolerance"):
                    nc.vector.reduce_sum(
                        dst_ap,
                        children_ap.rearrange("c (p e) -> c p e", e=8),
                        axis=AxX)

            def l0red(k):
                aggt = a0p.tile([C, PT], bf16, tag="agg0", name="aggt")
                ch = (xA0[:, S:NPAR1] if k == 0
                      else xh2t[:, 8 * PT * (k - 1): 8 * PT * k])
                red8(aggt[:, :], ch)
                return aggt

            def l0mm(k, aggt):
                ps0 = pp.tile([C, PT], f32, tag="ps")
                nc.tensor.matmul(ps0, w["w1a"], aggt[:, :],
                                 start=True, stop=False)
                nc.tensor.matmul(ps0, w["w1b"], xA0[:, PT * k: PT * (k + 1)],
                                 start=False, stop=True)
                nc.scalar.activation(h1self[:, PT * k: PT * (k + 1)], ps0,
                                     Relu, bias=bt["b1"])

            def l1mm(t, aggt):
                # layer-1 tile for parents [512t, 512(t+1)) -> h2, plus the
                # (agg-free) layer-2 self-only output for the same columns.
                ps1 = pp.tile([C, PT], f32, tag="ps")
                nc.tensor.matmul(ps1, w["w2a"], aggt[:, :],
                                 start=True, stop=False)
                nc.tensor.matmul(ps1, w["w2b"],
                                 h1self[:, PT * t: PT * (t + 1)],
                                 start=False, stop=True)
                nc.scalar.activation(h2sb[:, PT * t: PT * (t + 1)], ps1,
                                     Relu, bias=bt["b2"])
                if t > 0:
                    psn = pp.tile([C, PT], f32, tag="ps")
                    nc.tensor.matmul(psn, w["w3b"],
                                     h2sb[:, PT * t: PT * (t + 1)],
                                     start=True, stop=True)
                    nc.scalar.activation(ostage[:, PT * t: PT * (t + 1)],
                                         psn, Relu, bias=bt["b3"])

            # ---------------- software-pipelined main loop ----------------
            # hop2 chunk k is on-chip by block k-1, so layer-0 reductions
            # for tiles 2..8 run early; h1self is complete by block 7,
            # letting the layer-1 tile-0 work (children span ALL h1self
            # tiles) retire inside block 8 instead of the tail.
            L0RED = {1: [0, 1], 2: [2, 3], 3: [4], 4: [5], 5: [6], 6: [7],
                     7: [8]}
            l0_pending = {}      # k -> aggt awaiting its matmul
            l1_aggs = {}         # t -> per-block layer-1 agg tile
            l1a0 = None
            for t in range(1, 9):
                X3t = dpool.tile([C, 8 * 8 * PT], fp8, tag="X3")
                if t < 8:
                    nc.sync.dma_start(
                        X3t[:, :], x3[:, N3 // 8 * (t - 1): N3 // 8 * t])
                else:
                    # split the last chunk so block 8's compute (the tail
                    # drain) starts half a transfer earlier
                    nc.sync.dma_start(
                        X3t[:, : 4 * 8 * PT],
                        x3[:, N3 // 8 * 7: N3 // 8 * 7 + 4 * 8 * PT])
                    nc.sync.dma_start(
                        X3t[:, 4 * 8 * PT:],
                        x3[:, N3 // 8 * 7 + 4 * 8 * PT: N3])
                if t == 1:
                    nc.sync.dma_start(xA0[:, :], xA[:, :])
                    nc.sync.dma_start(xh2t[:, 8 * PT: 2 * 8 * PT],
                                      xh2d[:, 8 * PT: 2 * 8 * PT])
                elif t == 2:
                    # hop2 chunks 3-8: queued after X3(2) so it does not
                    # delay block 2's children; lands well before block 3.
                    nc.sync.dma_start(xh2t[:, 2 * 8 * PT: NH2],
                                      xh2d[:, 2 * 8 * PT: NH2])

                h1tmp = hpool.tile([C, 8 * PT], bf16, tag="h1tmp")
                agg1 = a1p.tile([C, PT], bf16, tag="agg1", name="agg1")
                l1_aggs[t] = agg1
                reds = list(L0RED.get(t, ()))
                for u in range(8):
                    # --- the big fp8 aggregation for 512 hop2 parents ---
                    psu = pp.tile([C, PT], f32, tag="ps")
                    cb = 8 * PT * u
                    self_ap = xh2t[:, 8 * PT * (t - 1) + PT * u:
                                   8 * PT * (t - 1) + PT * (u + 1)]
                    if dr:
                        for e in range(4):
                            rhs = X3t[:, cb + 2 * PT * e: cb + 2 * PT * (e + 1)]
                            nc.tensor.matmul(
                                psu, wdrt[:, :].rearrange("c (j m) -> c j m", j=2),
                                rhs.rearrange("c (j n) -> c j n", j=2),
                                start=(e == 0), stop=False, perf_mode=DRow)
                        nc.tensor.matmul(psu, w["w1bs"], self_ap,
                                         start=False, stop=True)
                        nc.scalar.activation(
                            h1tmp[:, PT * u: PT * (u + 1)], psu, Relu,
                            bias=bt["b1"], scale=1.0 / (8.0 * DR_SCALE))
                    else:
                        for e in range(8):
                            nc.tensor.matmul(
                                psu, w["w1a"],
                                X3t[:, cb + PT * e: cb + PT * (e + 1)],
                                start=(e == 0), stop=False)
                        nc.tensor.matmul(psu, w["w1b"], self_ap,
                                         start=False, stop=True)
                        nc.scalar.activation(
                            h1tmp[:, PT * u: PT * (u + 1)], psu, Relu,
                            bias=bt["b1"])
                    # --- layer-1 partial aggregation for these 64 parents ---
                    red8(agg1[:, 64 * u: 64 * (u + 1)],
                         h1tmp[:, PT * u: PT * (u + 1)])

                    # --- interleaved small work (deps satisfied earlier) ---
                    if u == 1 and reds:
                        l0_pending[reds[0]] = l0red(reds[0])
                    elif u == 2:
                        if t >= 2:
                            l1mm(t - 1, l1_aggs.pop(t - 1))
                        if t == 8:
                            l1mm(0, l1a0)
                    elif u == 3 and len(reds) > 1:
                        l0_pending[reds[1]] = l0red(reds[1])
                    elif u == 4 and t == 8:
                        # layer-2 agg, parents 0..447 (children in h2
                        # tiles 1..7, all written by now)
                        red8(l2agg[:, 0:448], h2sb[:, S: S + 8 * 448])
                    elif u == 6 and reds:
                        l0mm(reds[0], l0_pending.pop(reds[0]))
                    elif u == 7:
                        if len(reds) > 1:
                            l0mm(reds[1], l0_pending.pop(reds[1]))
                        if t == 7:
                            # layer-1 tile 0 agg: children = h1self tiles
                            # 1..8, the last of which was written at u6
                            l1a0 = a1p.tile([C, PT], bf16, tag="agg1",
                                            name="l1a0")
                            red8(l1a0[:, :], h1self[:, S:NPAR1])

            # ---------------- tail ----------------
            l1mm(8, l1_aggs.pop(8))
            # layer-2 agg, parents 448..511 (children in h2 tile 8)
            red8(l2agg[:, 448:512], h2sb[:, S + 8 * 448: NPAR1])
            # layer 2, parents [0, 512): full agg + self on h2 tile 0
            ps2 = pp.tile([C, PT], f32, tag="ps")
            nc.tensor.matmul(ps2, w["w3a"], l2agg[:, :],
                             start=True, stop=False)
            nc.tensor.matmul(ps2, w["w3b"], h2sb[:, 0:S],
                             start=False, stop=True)
            nc.scalar.activation(ostage[:, 0:S], ps2, Relu, bias=bt["b3"])
            nc.sync.dma_start(out[:, :], ostage[:, :])

    nc.compile()
    return nc


def _get_bass(dr, h2f8):
    key = (dr, h2f8)
    if key not in _BASS_CACHE:
        _BASS_CACHE[key] = _build_bass(dr, h2f8)
    return _BASS_CACHE[key]


def _edge_is_tree(edge):
    if edge.shape != (2, E_FULL):
        return False
    ar = np.arange(E_FULL, dtype=np.int64)
    return (np.array_equal(edge[0], (B + ar).astype(np.int32))
            and np.array_equal(edge[1], (ar // 8).astype(np.int32)))


def _fallback(x, edge, W1, b1, W2, b2, W3, b3):
    # General (structure-agnostic) CPU implementation; only used if the
    # inputs are not the fanout-8 tree this kernel is specialized for.
    sizes = [(N_FULL, E_FULL), (299008, 294912), (36864, 32768)]
    params = [(W1, b1), (W2, b2), (W3, b3)]
    x = x.astype(np.float32)
    for (n, e), (Wl, bl) in zip(sizes, params):
        src = edge[0, :e].astype(np.int64)
        dst = edge[1, :e].astype(np.int64)
        x = x[:n]
        agg = np.zeros((n, x.shape[1]), np.float32)
        np.add.at(agg, dst, x[src])
        deg = np.bincount(dst, minlength=n).astype(np.float32)
        agg /= np.maximum(deg, 1.0)[:, None]
        x = np.maximum(np.concatenate([agg, x], axis=1) @ Wl.T + bl, 0.0)
    return x


def kernel(**inputs):
    global LAST_RESULT
    import ml_dtypes

    x = np.asarray(inputs["x"])
    edge = np.asarray(inputs["edge"])
    W = [np.asarray(inputs[k], dtype=np.float32) for k in ("W1", "W2", "W3")]
    bias = [np.asarray(inputs[k], dtype=np.float32) for k in ("b1", "b2", "b3")]

    if x.shape != (N_FULL, C) or not _edge_is_tree(edge):
        return _fallback(x, edge, W[0], bias[0], W[1], bias[1], W[2], bias[2])

    from concourse.bass_utils import run_bass_kernel_spmd

    bf = ml_dtypes.bfloat16
    f8 = ml_dtypes.float8_e4m3fn          # bit-compatible with TRN e4m3 < 240
    x = np.ascontiguousarray(x, dtype=np.float32)

    wblocks = []
    for li in range(3):
        wblocks.append((W[li][:, :C] / 8.0).T)     # agg part, mean folded in
        wblocks.append(W[li][:, C:].T)             # self part
    # DoubleRow path: the fp8 agg weight (built on-device from w1a_raw,
    # scaled by DR_SCALE to sit in e4m3's normal range) pairs with a self
    # weight scaled by 8*DR_SCALE; the PSUM is divided back by 8*DR_SCALE
    # in the activation (ReLU is positively homogeneous), which also
    # restores the /8 of the mean.
    wblocks.append(W[0][:, C:].T * (8.0 * DR_SCALE))          # w1bs
    wblocks.append(W[0][:, :C].T)                             # w1a_raw
    wblocks.append(np.stack(bias, axis=1))                    # 3 bias cols
    wconsts = np.ascontiguousarray(np.concatenate(wblocks, axis=1)).astype(bf)

    h2np = f8 if H2F8 else bf
    in_maps = []
    for c in range(N_CORES):
        xloc = [x[OFF[h] + BLK[h] * c: OFF[h] + BLK[h] * (c + 1)]
                for h in range(4)]
        xAc = np.ascontiguousarray(np.concatenate(xloc[:2], axis=0).T).astype(bf)
        xh2c = np.ascontiguousarray(xloc[2].T).astype(h2np)
        # de-interleave hop3 per 512-parent group: within each 4096-row
        # chunk, row e*512 + p  <-  child e of parent p (old row 8p + e)
        x3 = xloc[3].reshape(-1, PT, 8, C).transpose(0, 2, 1, 3).reshape(-1, C)
        x3c = np.ascontiguousarray(x3.T).astype(f8)
        in_maps.append({"xA": xAc, "xh2": xh2c, "x3": x3c,
                        "wconsts": wconsts})

    nc = _get_bass(DR, H2F8)
    res = run_bass_kernel_spmd(nc, in_maps, list(range(N_CORES)), trace=TRACE)
    LAST_RESULT = res

    out = np.empty((OUT_ROWS, C), np.float32)
    for c in range(N_CORES):
        oc = np.asarray(res.results[c]["out"]).astype(np.float32)
        out[S * c: S * (c + 1)] = oc[:, :S].T
        out[B + 8 * S * c: B + 8 * S * (c + 1)] = oc[:, S:].T
    return out


# revision 24
# speedup vs baseline: 1.0094x; 1.0094x over previous
"""Trainium2 Bass kernel for 3-layer CuGraphSAGE on a fanout-8 sampled tree.

The sampled graph produced by fanout-based neighbor sampling is a forest of
B=4096 independent trees (children of parent p are rows [4096+8p, 4096+8p+8)).
We shard by seed block: core c gets 512 seeds plus their full 3-hop subtrees
(4 contiguous row blocks of x, exactly 1/8 of all rows, zero halo).

Per-core pipeline (all activations channel-major [128ch, rows] so the matmul
contraction dim is always the partition dim — no transposes on device):
  hop3 (87.5% of bytes, only ever aggregated) streams as fp8_e4m3 — the
  8-way mean dilutes its 3.6% quantization RMS to ~1e-3 of the final
  output.  It is de-interleaved per 512-parent group on the host (col
  e*512+p = child e of parent p), so the mean-aggregation is accumulating
  matmuls with CONTIGUOUS fp8 rhs slices (stride-8 APs stream ~5x slower
  on the PE); with GNN_DR=1 pairs of sibling planes go through fp8
  DoubleRow matmuls (2 MACs/cell/cycle), halving PE streaming time.
  hop2 is resident fp8 (self-features quantize at 3.6% RMS -> ~1e-2
  final); hop0+hop1 (whose rows dominate the output) stay bf16.  The
  small hop1/hop2 and layer-1/2 aggregations run as VectorE reduce_sum
  (DVE is otherwise idle) followed by one matmul, software-pipelined so
  they hide inside the big fp8 u-blocks.  DMA count is minimized (per-
  transfer overhead ~3us is serial): one weights+bias transfer, one
  hop0/1, two hop2, eight 4 MiB hop3, one staged output write-back.
  The 1/8 mean is folded into the aggregation weight (via the activation
  scale on the DoubleRow path, whose fp8 weights are built on-device);
  bias+ReLU on ScalarE evicts PSUM->SBUF in bf16.  h1/h2 live entirely
  in SBUF; the bf16 output is upcast to fp32 on the host.
"""

import os
import numpy as np

# ---------------------------------------------------------------- constants
N_CORES = 8
C = 128                       # channels
B = 4096                      # seeds
S = B // N_CORES              # 512 seeds per core
BLK = [512, 4096, 32768, 262144]          # per-core rows per hop
OFF = [0, 4096, 36864, 299008]            # global start row of each hop block
NPAR0 = BLK[0] + BLK[1] + BLK[2]          # 37376 local layer-0 parents
NPAR1 = BLK[0] + BLK[1]                   # 4608 local layer-1 parents
NH2 = BLK[2]                              # 32768 local hop2 rows
N3 = BLK[3]                               # 262144 local hop3 rows
PT = 512                                  # parents per PSUM tile
N_FULL = 2396160
E_FULL = 2392064
OUT_ROWS = 36864
DR_SCALE = 16.0               # fp8 DoubleRow agg-weight scale (see below)

TRACE = os.environ.get("GNN_TRACE", "0") == "1"
DR = os.environ.get("GNN_DR", "1") == "1"
H2F8 = os.environ.get("GNN_H2F8", "1") == "1"
LAST_RESULT = None

_BASS_CACHE = {}


def _build_bass(dr, h2f8):
    import concourse.mybir as mybir
    from concourse import bacc
    from concourse.tile import TileContext

    bf16 = mybir.dt.bfloat16
    fp8 = mybir.dt.float8e4
    f32 = mybir.dt.float32
    h2dt = fp8 if h2f8 else bf16
    Relu = mybir.ActivationFunctionType.Relu
    Ident = mybir.ActivationFunctionType.Identity
    AxX = mybir.AxisListType.X
    DRow = mybir.MatmulPerfMode.DoubleRow

    # Bacc (not raw Bass): its compile() pipeline splits multi-sem sync
    # waits into event semaphores — TRN2 allows at most 1 wait/instruction.
    nc = bacc.Bacc()
    xA = nc.dram_tensor("xA", [C, NPAR1], bf16, kind="ExternalInput")
    xh2d = nc.dram_tensor("xh2", [C, NH2], h2dt, kind="ExternalInput")
    x3 = nc.dram_tensor("x3", [C, N3], fp8, kind="ExternalInput")
    # 8 weight blocks + 3 bias columns in one bf16 tensor -> one DMA
    # (per-transfer overhead is serial on the DMA timeline)
    wconsts = nc.dram_tensor("wconsts", [C, 8 * C + 3], bf16,
                             kind="ExternalInput")
    out = nc.dram_tensor("out", [C, NPAR1], bf16, kind="ExternalOutput")
    WIDX = {k: i for i, k in enumerate(
        ("w1a", "w1b", "w2a", "w2b", "w3a", "w3b", "w1bs", "w1a_raw"))}

    with TileContext(nc) as tc:
        with tc.tile_pool(name="const", bufs=1) as constp, \
             tc.tile_pool(name="keep", bufs=1) as keepp, \
             tc.tile_pool(name="dbuf", bufs=2) as dpool, \
             tc.tile_pool(name="hbuf", bufs=2) as hpool, \
             tc.tile_pool(name="a0buf", bufs=2) as a0p, \
             tc.tile_pool(name="a1buf", bufs=3) as a1p, \
             tc.tile_pool(name="ps", bufs=6, space="PSUM") as pp:

            wtile = constp.tile([C, 8 * C + 3], bf16, name="wtile")
            nc.sync.dma_start(wtile[:, :], wconsts[:, :])
            w = {k: wtile[:, C * i: C * (i + 1)] for k, i in WIDX.items()}
            bt = {f"b{i+1}": wtile[:, 8 * C + i: 8 * C + i + 1]
                  for i in range(3)}
            # fp8 DoubleRow aggregation weight, built on-device: two
            # interleaved copies of DR_SCALE * W1a^T (saves a DMA transfer)
            wdrt = constp.tile([C, 2 * C], fp8, name="wdrt")
            with nc.allow_low_precision(
                    reason="fp8 DoubleRow agg weights; 8-way mean dilutes "
                           "the 3.6% fp8 RMS below tolerance"):
                nc.scalar.activation(wdrt[:, 0:C], w["w1a_raw"], Ident,
                                     scale=DR_SCALE)
                nc.scalar.activation(wdrt[:, C:2 * C], w["w1a_raw"], Ident,
                                     scale=DR_SCALE)

            xA0 = keepp.tile([C, NPAR1], bf16, tag="xA0")
            nc.sync.dma_start(xA0[:, :], xA[:, :])
            # hop2: resident, two transfers (blocks 1-2 need chunks 1-2
            # early; the rest streams during block 1)
            xh2t = keepp.tile([C, NH2], h2dt, tag="xh2t")
            nc.sync.dma_start(xh2t[:, 0: 2 * 8 * PT], xh2d[:, 0: 2 * 8 * PT])

            h1self = keepp.tile([C, NPAR1], bf16, tag="h1self")
            h2sb = keepp.tile([C, NPAR1], bf16, tag="h2sb")
            l2agg = keepp.tile([C, PT], bf16, tag="l2agg")
            ostage = keepp.tile([C, NPAR1], bf16, tag="ostage")

            def red8(dst_ap, children_ap):
                # dst[c, p] = sum_e children[c, 8p+e]  (natural node order)
                with nc.allow_low_precision(
                        reason="8-way sibling sum is fp32 internal on DVE; "
                               "bf16 rounding of the sum is within tolerance"):
                    nc.vector.reduce_sum(
                        dst_ap,
                        children_ap.rearrange("c (p e) -> c p e", e=8),
                        axis=AxX)

            def l0red(k):
                aggt = a0p.tile([C, PT], bf16, tag="agg0", name="aggt")
                ch = (xA0[:, S:NPAR1] if k == 0
                      else xh2t[:, 8 * PT * (k - 1): 8 * PT * k])
                red8(aggt[:, :], ch)
                return aggt

            def l0mm(k, aggt):
                ps0 = pp.tile([C, PT], f32, tag="ps")
                nc.tensor.matmul(ps0, w["w1a"], aggt[:, :],
                                 start=True, stop=False)
                nc.tensor.matmul(ps0, w["w1b"], xA0[:, PT * k: PT * (k + 1)],
                                 start=False, stop=True)
                nc.scalar.activation(h1self[:, PT * k: PT * (k + 1)], ps0,
                                     Relu, bias=bt["b1"])

            def l1mm(t, aggt):
                # layer-1 tile for parents [512t, 512(t+1)) -> h2, plus the
                # (agg-free) layer-2 self-only output for the same columns.
                ps1 = pp.tile([C, PT], f32, tag="ps")
                nc.tensor.matmul(ps1, w["w2a"], aggt[:, :],
                                 start=True, stop=False)
                nc.tensor.matmul(ps1, w["w2b"],
                                 h1self[:, PT * t: PT * (t + 1)],
                                 start=False, stop=True)
                nc.scalar.activation(h2sb[:, PT * t: PT * (t + 1)], ps1,
                                     Relu, bias=bt["b2"])
                if t > 0:
                    psn = pp.tile([C, PT], f32, tag="ps")
                    nc.tensor.matmul(psn, w["w3b"],
                                     h2sb[:, PT * t: PT * (t + 1)],
                                     start=True, stop=True)
                    nc.scalar.activation(ostage[:, PT * t: PT * (t + 1)],
                                         psn, Relu, bias=bt["b3"])

            # ---------------- software-pipelined main loop ----------------
            # hop2 chunk k is on-chip by block k-1, so layer-0 reductions
            # for tiles 2..8 run early; h1self is complete by block 7,
            # letting the layer-1 tile-0 work (children span ALL h1self
            # tiles) retire inside block 8 instead of the tail.
            L0RED = {1: [0, 1], 2: [2, 3], 3: [4], 4: [5], 5: [6], 6: [7],
                     7: [8]}
            l0_pending = {}      # k -> aggt awaiting its matmul
            l1_aggs = {}         # t -> per-block layer-1 agg tile
            l1a0 = None
            for t in range(1, 9):
                X3t = dpool.tile([C, 8 * 8 * PT], fp8, tag="X3")
                nc.sync.dma_start(
                    X3t[:, :], x3[:, N3 // 8 * (t - 1): N3 // 8 * t])
                if t == 2:
                    # hop2 chunks 3-8: queued after X3(2) so it does not
                    # delay block 2's children; lands well before block 3.
                    nc.sync.dma_start(xh2t[:, 2 * 8 * PT: NH2],
                                      xh2d[:, 2 * 8 * PT: NH2])

                h1tmp = hpool.tile([C, 8 * PT], bf16, tag="h1tmp")
                agg1 = a1p.tile([C, PT], bf16, tag="agg1", name="agg1")
                l1_aggs[t] = agg1
                reds = list(L0RED.get(t, ()))
                if t == 1:
                    # prologue: seeds' layer-0 tile needs only xA0 — fill
                    # the PE/DVE idle time while the first X3 chunk streams.
                    pr = l0red(0)
                    l0mm(0, pr)
                    reds.remove(0)
                for u in range(8):
                    # --- the big fp8 aggregation for 512 hop2 parents ---
                    psu = pp.tile([C, PT], f32, tag="ps")
                    cb = 8 * PT * u
                    self_ap = xh2t[:, 8 * PT * (t - 1) + PT * u:
                                   8 * PT * (t - 1) + PT * (u + 1)]
                    if dr:
                        for e in range(4):
                            rhs = X3t[:, cb + 2 * PT * e: cb + 2 * PT * (e + 1)]
                            nc.tensor.matmul(
                                psu, wdrt[:, :].rearrange("c (j m) -> c j m", j=2),
                                rhs.rearrange("c (j n) -> c j n", j=2),
                                start=(e == 0), stop=False, perf_mode=DRow)
                        nc.tensor.matmul(psu, w["w1bs"], self_ap,
                                         start=False, stop=True)
                        nc.scalar.activation(
                            h1tmp[:, PT * u: PT * (u + 1)], psu, Relu,
                            bias=bt["b1"], scale=1.0 / (8.0 * DR_SCALE))
                    else:
                        for e in range(8):
                            nc.tensor.matmul(
                                psu, w["w1a"],
                                X3t[:, cb + PT * e: cb + PT * (e + 1)],
                                start=(e == 0), stop=False)
                        nc.tensor.matmul(psu, w["w1b"], self_ap,
                                         start=False, stop=True)
                        nc.scalar.activation(
                            h1tmp[:, PT * u: PT * (u + 1)], psu, Relu,
                            bias=bt["b1"])
                    # --- layer-1 partial aggregation for these 64 parents ---
                    red8(agg1[:, 64 * u: 64 * (u + 1)],
                         h1tmp[:, PT * u: PT * (u + 1)])

                    # --- interleaved small work (deps satisfied earlier) ---
                    if u == 1 and reds:
                        l0_pending[reds[0]] = l0red(reds[0])
                    elif u == 2:
                        if t >= 2:
                            l1mm(t - 1, l1_aggs.pop(t - 1))
                        if t == 8:
                            l1mm(0, l1a0)
                    elif u == 3 and len(reds) > 1:
                        l0_pending[reds[1]] = l0red(reds[1])
                    elif u == 4 and t == 8:
                        # layer-2 agg, parents 0..447 (children in h2
                        # tiles 1..7, all written by now)
                        red8(l2agg[:, 0:448], h2sb[:, S: S + 8 * 448])
                    elif u == 6 and reds:
                        l0mm(reds[0], l0_pending.pop(reds[0]))
                    elif u == 7:
                        if len(reds) > 1:
                            l0mm(reds[1], l0_pending.pop(reds[1]))
                        if t == 7:
                            # layer-1 tile 0 agg: children = h1self tiles
                            # 1..8, the last of which was written at u6
                            l1a0 = a1p.tile([C, PT], bf16, tag="agg1",
                                            name="l1a0")
                            red8(l1a0[:, :], h1self[:, S:NPAR1])

            # ---------------- tail ----------------
            l1mm(8, l1_aggs.pop(8))
            # layer-2 agg, parents 448..511 (children in h2 tile 8)
            red8(l2agg[:, 448:512], h2sb[:, S + 8 * 448: NPAR1])
            # layer 2, parents [0, 512): full agg + self on h2 tile 0
            ps2 = pp.tile([C, PT], f32, tag="ps")
            nc.tensor.matmul(ps2, w["w3a"], l2agg[:, :],
                             start=True, stop=False)
            nc.tensor.matmul(ps2, w["w3b"], h2sb[:, 0:S],
                             start=False, stop=True)
            nc.scalar.activation(ostage[:, 0:S], ps2, Relu, bias=bt["b3"])
            nc.sync.dma_start(out[:, :], ostage[:, :])

    nc.compile()
    return nc


def _get_bass(dr, h2f8):
    key = (dr, h2f8)
    if key not in _BASS_CACHE:
        _BASS_CACHE[key] = _build_bass(dr, h2f8)
    return _BASS_CACHE[key]


def _edge_is_tree(edge):
    if edge.shape != (2, E_FULL):
        return False
    ar = np.arange(E_FULL, dtype=np.int64)
    return (np.array_equal(edge[0], (B + ar).astype(np.int32))
            and np.array_equal(edge[1], (ar // 8).astype(np.int32)))


def _fallback(x, edge, W1, b1, W2, b2, W3, b3):
    # General (structure-agnostic) CPU implementation; only used if the
    # inputs are not the fanout-8 tree this kernel is specialized for.
    sizes = [(N_FULL, E_FULL), (299008, 294912), (36864, 32768)]
    params = [(W1, b1), (W2, b2), (W3, b3)]
    x = x.astype(np.float32)
    for (n, e), (Wl, bl) in zip(sizes, params):
        src = edge[0, :e].astype(np.int64)
        dst = edge[1, :e].astype(np.int64)
        x = x[:n]
        agg = np.zeros((n, x.shape[1]), np.float32)
        np.add.at(agg, dst, x[src])
        deg = np.bincount(dst, minlength=n).astype(np.float32)
        agg /= np.maximum(deg, 1.0)[:, None]
        x = np.maximum(np.concatenate([agg, x], axis=1) @ Wl.T + bl, 0.0)
    return x


def kernel(**inputs):
    global LAST_RESULT
    import ml_dtypes

    x = np.asarray(inputs["x"])
    edge = np.asarray(inputs["edge"])
    W = [np.asarray(inputs[k], dtype=np.float32) for k in ("W1", "W2", "W3")]
    bias = [np.asarray(inputs[k], dtype=np.float32) for k in ("b1", "b2", "b3")]

    if x.shape != (N_FULL, C) or not _edge_is_tree(edge):
        return _fallback(x, edge, W[0], bias[0], W[1], bias[1], W[2], bias[2])

    from concourse.bass_utils import run_bass_kernel_spmd

    bf = ml_dtypes.bfloat16
    f8 = ml_dtypes.float8_e4m3fn          # bit-compatible with TRN e4m3 < 240
    x = np.ascontiguousarray(x, dtype=np.float32)

    wblocks = []
    for li in range(3):
        wblocks.append((W[li][:, :C] / 8.0).T)     # agg part, mean folded in
        wblocks.append(W[li][:, C:].T)             # self part
    # DoubleRow path: the fp8 agg weight (built on-device from w1a_raw,
    # scaled by DR_SCALE to sit in e4m3's normal range) pairs with a self
    # weight scaled by 8*DR_SCALE; the PSUM is divided back by 8*DR_SCALE
    # in the activation (ReLU is positively homogeneous), which also
    # restores the /8 of the mean.
    wblocks.append(W[0][:, C:].T * (8.0 * DR_SCALE))          # w1bs
    wblocks.append(W[0][:, :C].T)                             # w1a_raw
    wblocks.append(np.stack(bias, axis=1))                    # 3 bias cols
    wconsts = np.ascontiguousarray(np.concatenate(wblocks, axis=1)).astype(bf)

    h2np = f8 if H2F8 else bf
    in_maps = []
    for c in range(N_CORES):
        xloc = [x[OFF[h] + BLK[h] * c: OFF[h] + BLK[h] * (c + 1)]
                for h in range(4)]
        xAc = np.ascontiguousarray(np.concatenate(xloc[:2], axis=0).T).astype(bf)
        xh2c = np.ascontiguousarray(xloc[2].T).astype(h2np)
        # de-interleave hop3 per 512-parent group: within each 4096-row
        # chunk, row e*512 + p  <-  child e of parent p (old row 8p + e)
        x3 = xloc[3].reshape(-1, PT, 8, C).transpose(0, 2, 1, 3).reshape(-1, C)
        x3c = np.ascontiguousarray(x3.T).astype(f8)
        in_maps.append({"xA": xAc, "xh2": xh2c, "x3": x3c,
                        "wconsts": wconsts})

    nc = _get_bass(DR, H2F8)
    res = run_bass_kernel_spmd(nc, in_maps, list(range(N_CORES)), trace=TRACE)
    LAST_RESULT = res

    out = np.empty((OUT_ROWS, C), np.float32)
    for c in range(N_CORES):
        oc = np.asarray(res.results[c]["out"]).astype(np.float32)
        out[S * c: S * (c + 1)] = oc[:, :S].T
        out[B + 8 * S * c: B + 8 * S * (c + 1)] = oc[:, S:].T
    return out
